# revision 1
# baseline (speedup 1.0000x reference)
"""Trainium2 Bass kernel for the ETD1 ODE block (nn_ODEblockW_28922309771809).

Math (mirrors the jax reference, but solve-free):
  s    = 0.05 * sigmoid(alpha)                       # row scales (0.5*dt)
  X    = dt*A = diag(s) @ (adj - I)                  # [2048,2048], ||X|| ~ 0.073
  m1_L = e^X     via degree-8 Taylor, Paterson-Stockmeyer with Y = X^3
  m2   = A^{-1}(e^X - I) = dt*phi1(X),  phi1 = sum_k X^k/(k+1)!   (degree-8 PS)
  B    = (w*clip(d,0,1)) @ w.T - I  (symmetric);  Xr = dt*B;  m1_R = e^{Xr}
  F    = m2 @ x0
  z    = IC after 9 steps of IC <- m1_L @ IC @ m1_R + F   (N_STEPS = int(1.0//0.1) == 9)

Distribution over 8 cores (transposed-column-local formulation):
  The node dim (2048) is sharded 256 rows/core; every local tensor is held as
  the transposed column block [2048|1024, 256], so each big matmul is
     out_colT[m] = sum_k  matmul(lhsT = Full[kblk, mblk] from DRAM, rhs = colT[kblk])
  Full matrices are assembled by AllGather of row blocks (PE-transpose of the
  local column block first). The feature dim (1024) is sharded 128/core.

  AllGathered tensors use a TILED layout: each rank's contribution is a
  sequence of [128,128] tiles (m-major), so the per-m lhsT slab loads read
  8-16 contiguous 32-64KB blocks instead of 256B-strided rows. Node-dim
  gathers are split into two pipelined half-gathers (half j carries k-chunks
  k%2==j); consuming matmuls run even k-chunks first so they start as soon as
  the first half lands.

Precision: series matmuls in bf16 (bf16 error only enters quadratic+ Taylor
terms of e^X; the I and X terms are exact fp32 elementwise), recurrence /
forcing / R-side matmuls in float32r. Measured ~9.5e-4 frob rel err vs the
fp32 reference, which itself carries ~1.7e-4 fp32 rounding noise vs fp64.
"""

import math
from contextlib import ExitStack

import numpy as np

import concourse.bass as bass
import concourse.mybir as mybir
import concourse.tile as tile
from concourse import bacc
from concourse.bass_utils import run_bass_kernel_spmd
from concourse.masks import make_identity

F32 = mybir.dt.float32
F32R = mybir.dt.float32r
BF16 = mybir.dt.bfloat16
AL = mybir.AluOpType

N_CORES = 8
P = 128
N = 2048          # nodes
D = 1024          # features
RB = 256          # node row-block per core
FB = 256          # node col-block width (L side)
FBR = 128         # feature block width (R side, true 8-way shard)
NKC = N // P      # 16
DKC = D // P      # 8
RJ = RB // P      # 2
NSTEPS = 9        # int(1.0 // 0.1) == 9

EC = [1.0 / math.factorial(k) for k in range(9)]        # e^X coeffs
PC = [0.1 / math.factorial(k + 1) for k in range(9)]    # dt*phi1(X) coeffs

LGROUP = [list(range(N_CORES))]


def build_nc():
    nc = bacc.Bacc("TRN2", target_bir_lowering=False, debug=False,
                   num_devices=N_CORES)

    # ---- I/O (per-core shards fed host-side; same NEFF on all cores) ----
    adj_rows = nc.dram_tensor("adj_rows", [RB, N], F32, kind="ExternalInput")
    eye_rows = nc.dram_tensor("eye_rows", [RB, N], F32, kind="ExternalInput")
    eye_colT = nc.dram_tensor("eye_colT", [N, RB], F32, kind="ExternalInput")
    alpha_blk = nc.dram_tensor("alpha_blk", [RB], F32, kind="ExternalInput")
    x_full = nc.dram_tensor("x_full", [N, D], F32, kind="ExternalInput")
    x0_full = nc.dram_tensor("x0_full", [N, D], F32, kind="ExternalInput")
    w_cols = nc.dram_tensor("w_cols", [D, FBR], F32, kind="ExternalInput")
    w_rows = nc.dram_tensor("w_rows", [FBR, D], F32, kind="ExternalInput")
    eye_feat = nc.dram_tensor("eye_feat", [D, FBR], F32, kind="ExternalInput")
    d_full = nc.dram_tensor("d_full", [D], F32, kind="ExternalInput")
    z_loc = nc.dram_tensor("z_loc", [RB, D], F32, kind="ExternalOutput")

    with tile.TileContext(nc) as tc, ExitStack() as top:
        const = top.enter_context(tc.tile_pool(name="const", bufs=1))
        dram = top.enter_context(tc.tile_pool(name="dram", bufs=1, space="DRAM"))
        psum = top.enter_context(tc.tile_pool(name="psum", bufs=2, space="PSUM"))
        slabp = top.enter_context(tc.tile_pool(name="slabp", bufs=1))
        scrp = top.enter_context(tc.tile_pool(name="scrp", bufs=1))
        lser = top.enter_context(tc.tile_pool(name="lser", bufs=1))
        lout = top.enter_context(tc.tile_pool(name="lout", bufs=1))

        ident = const.tile([P, P], F32)
        make_identity(nc, ident)
        ident_b = const.tile([P, P], BF16)
        nc.vector.tensor_copy(ident_b[:], ident[:])

        def pe_t(dst_slice, src_slice, bf=False):
            """dst[128,128] = src[128,128].T via PE transpose."""
            if src_slice.dtype == F32R:
                src_slice = src_slice.bitcast(F32)
            ps = psum.tile([P, P], BF16 if bf else F32, tag="tr", bufs=4, name="ps_tr")
            nc.tensor.transpose(ps[:], src_slice, ident_b[:] if bf else ident[:])
            nc.vector.tensor_copy(dst_slice, ps[:])

        def combo(dst_slice, eye_m, xt_m, x2t_m, c0, c1, c2):
            """dst = c0*I + c1*X + c2*X2 for one [128,w] chunk."""
            if xt_m.dtype == F32R:
                xt_m = xt_m.bitcast(F32)
            if x2t_m.dtype == F32R:
                x2t_m = x2t_m.bitcast(F32)
            w = xt_m.shape[-1]
            st = scrp.tile([P, FB], F32, tag="combo", bufs=3, name="combo_scr")
            s = st[:, :w]
            nc.vector.tensor_scalar_mul(s, xt_m, c1)
            nc.vector.scalar_tensor_tensor(s, x2t_m, c2, s, AL.mult, AL.add)
            nc.vector.scalar_tensor_tensor(dst_slice, eye_m, c0, s, AL.mult, AL.add)

        def load_eye(dram_t, m, w=FB):
            t = scrp.tile([P, FB], F32, tag="eye", bufs=2, name="eye_chunk")
            nc.sync.dma_start(t[:, :w], dram_t[m * P:(m + 1) * P, :])
            return t[:, :w]

        def _bc(src_ap, dt):
            if dt == F32R and src_ap.dtype == F32:
                return src_ap.bitcast(F32R)
            return src_ap

        # ---- tiled-gather helpers -------------------------------------
        # A gathered tensor is [ranks * tiles * P, P]: rank c's contribution
        # is `tiles` contiguous [128,128] tiles (tile t = cols t*128 of the
        # rank's [128, tiles*128] row block).
        def put_tiles(ccin, row_sb, tiles):
            """DMA row block row_sb [128, tiles*128] into tiled ccin."""
            for t in range(tiles):
                nc.sync.dma_start(ccin[t * P:(t + 1) * P, :],
                                  row_sb[:, t * P:(t + 1) * P])

        def tiled_src(g, m, jpr, tiles, dt):
            """AP over gathered g: [128, ranks, jpr, 128] = tile m of every
            rank's jpr row-chunks. Contribution tile order: j-major, m-minor."""
            a = _bc(g[:], dt).rearrange("(c j t p) n -> p c j t n", c=N_CORES,
                                        j=jpr, t=tiles, p=P)
            return a[:, :, :, m, :]

        def gather_tiled(produce, jpr, tiles, dt, name):
            """Single AllGather with tiled contribution: jpr row-chunks of
            `tiles` [128,128] tiles each. produce(ccin) fills it."""
            ccin = dram.tile([jpr * tiles * P, P], dt, tag=f"ccin_{name}",
                             name=f"ccin_{name}")
            full = dram.tile([N_CORES * jpr * tiles * P, P], dt,
                             addr_space="Shared", name=f"full_{name}")
            produce(ccin)
            nc.gpsimd.collective_compute(
                "AllGather", AL.bypass, replica_groups=LGROUP,
                ins=[ccin.opt()], outs=[full.opt()])
            return full

        def mm_pass(rhs_tiles, n_k, n_m, evict, dt, tag, nb=FB,
                    g=None, jpr=1, plain=None, tiles=None):
            """For each output chunk m: psums[i] = sum_k lhsT[k,m].T @ rhs[i][k].

            lhsT source: either `plain` (a [n_k*P, n_m*P] DRAM AP, k-chunk k at
            rows k*128) or `g` (a tiled-gathered tensor where k-chunk k lives
            as rank k//jpr, row-chunk k%jpr). For dt == F32R the rhs tiles
            must already be float32r-dtyped."""
            tiles_ = n_m if tiles is None else tiles
            for m in range(n_m):
                if g is not None:
                    sl = slabp.tile([P, N_CORES, jpr, P], dt, tag=tag,
                                    bufs=2, name=f"slab_{tag}")
                    nc.sync.dma_start(sl[:], tiled_src(g, m, jpr, tiles_, dt))
                    lt = lambda k: sl[:, k // jpr, k % jpr, :]
                else:
                    sl = slabp.tile([P, n_k, P], dt, tag=tag, bufs=2,
                                    name=f"slab_{tag}")
                    src = _bc(plain[:, m * P:(m + 1) * P], dt)
                    nc.sync.dma_start(sl[:], src.rearrange("(k p) n -> p k n", p=P))
                    lt = lambda k: sl[:, k, :]
                pss = [psum.tile([P, nb], F32, tag=f"mm{i}", bufs=2,
                                 name=f"ps_mm{i}") for i in range(len(rhs_tiles))]
                for k in range(n_k):
                    for ps, rhs in zip(pss, rhs_tiles):
                        nc.tensor.matmul(ps[:], lt(k), rhs[:, k, :],
                                         start=(k == 0), stop=(k == n_k - 1))
                evict(m, pss)

        # =========================================================
        # Prep scales
        # =========================================================
        s_sb = const.tile([P, RJ], F32)
        nc.sync.dma_start(s_sb[:], alpha_blk.ap().rearrange("(j p) -> p j", p=P))
        nc.scalar.activation(s_sb[:], s_sb[:], mybir.ActivationFunctionType.Sigmoid)
        nc.vector.tensor_scalar_mul(s_sb[:], s_sb[:], 0.05)

        d_sb = const.tile([P, DKC], F32)
        nc.sync.dma_start(d_sb[:], d_full.ap().rearrange("(q p) -> p q", p=P))
        nc.vector.tensor_scalar(d_sb[:], d_sb[:], 0.0, 1.0, AL.max, AL.min)

        xt = lser.tile([P, NKC, FB], F32)     # X^T col block, fp32
        x2t = lser.tile([P, NKC, FB], F32)    # (X^2)^T col block, fp32
        et = lout.tile([P, NKC, FB], F32R)    # m1_L^T col block
        m2t = lout.tile([P, NKC, FB], F32R)   # m2^T col block

        # =========================================================
        # Emission order interleaves the R-side (feature dim) chain between
        # the L-side passes: engine queues are in-order, so each R compute
        # segment is emitted one L-pass after the gather it depends on —
        # its semaphore wait is satisfied by the time the PE reaches it.
        # =========================================================
        pa_st, pr_st = ExitStack(), ExitStack()
        pr = pr_st.enter_context(tc.tile_pool(name="ph_r", bufs=1))
        pa = pa_st.enter_context(tc.tile_pool(name="ph_a", bufs=1))

        # --- R prep: w^T row block -> AllGather (earliest collective) ---
        wt_rowblk = pr.tile([P, D], F32)
        for k in range(DKC):
            wc_sb = pr.tile([P, FBR], F32, tag="w_in", bufs=2, name="wc_sb")
            nc.sync.dma_start(wc_sb[:], w_cols[k * P:(k + 1) * P, :])
            pe_t(wt_rowblk[:, k * P:(k + 1) * P], wc_sb[:])
        wt_g = gather_tiled(lambda ccin: put_tiles(ccin, wt_rowblk[:], DKC),
                            1, DKC, F32, "wt")

        # V = diag(clip(d)) @ w^T[:, Fblk]   [1024, 128]
        vr = pr.tile([P, DKC, FBR], F32R)
        wr_sb = pr.tile([P, D], F32, name="wr_sb")
        nc.sync.dma_start(wr_sb[:], w_rows[:])
        for k in range(DKC):
            pe_t(vr[:, k, :], wr_sb[:, k * P:(k + 1) * P])
        for k in range(DKC):
            nc.vector.tensor_scalar_mul(vr[:, k, :], vr[:, k, :].bitcast(F32),
                                        d_sb[:, k:k + 1])

        # --- Phase A: build X row block, AllGather X (bf16), transpose ---
        xrow = pa.tile([P, RJ, N], F32)
        xrow_b = pa.tile([P, RJ, N], BF16)
        ccin_x = dram.tile([RJ * NKC * P, P], BF16, name="ccin_x")
        for j in range(RJ):
            adj_sb = pa.tile([P, N], F32, tag="a_in", bufs=2, name="adj_sb")
            eyer_sb = pa.tile([P, N], F32, tag="a_in", bufs=2, name="eyer_sb")
            nc.sync.dma_start(adj_sb[:], adj_rows[j * P:(j + 1) * P, :])
            nc.sync.dma_start(eyer_sb[:], eye_rows[j * P:(j + 1) * P, :])
            nc.vector.tensor_sub(adj_sb[:], adj_sb[:], eyer_sb[:])
            nc.vector.tensor_scalar_mul(xrow[:, j, :], adj_sb[:], s_sb[:, j:j + 1])
            nc.vector.tensor_copy(xrow_b[:, j, :], xrow[:, j, :])
            put_tiles(ccin_x[j * NKC * P:(j + 1) * NKC * P, :], xrow_b[:, j, :], NKC)
        xfull_g = dram.tile([N_CORES * RJ * NKC * P, P], BF16,
                            addr_space="Shared", name="full_x")
        nc.gpsimd.collective_compute(
            "AllGather", AL.bypass, replica_groups=LGROUP,
            ins=[ccin_x.opt()], outs=[xfull_g.opt()])

        for k in range(NKC):
            for j in range(RJ):
                pe_t(xt[:, k, j * P:(j + 1) * P], xrow[:, j, k * P:(k + 1) * P])
        pa_st.close()

        # --- R: w_mat col block -> Xr = 0.1*(w_mat - I); gather Xr ---
        xr_col = pr.tile([P, DKC, FBR], F32R)

        def ev_wmat(m, pss):
            eyef = load_eye(eye_feat, m, FBR)
            nc.vector.tensor_sub(xr_col[:, m, :], pss[0][:], eyef)
            nc.vector.tensor_scalar_mul(xr_col[:, m, :],
                                        xr_col[:, m, :].bitcast(F32), 0.1)
        mm_pass([vr], DKC, DKC, ev_wmat, F32R, "fslab", nb=FBR,
                g=wt_g, tiles=DKC)

        def gather_sym(col_tile, name):
            """Symmetric [D,D] matrix: transpose col block -> row block -> AG."""
            rowblk = pr.tile([P, D], F32, tag="r_rowblk", bufs=2,
                             name=f"rowblk_{name}")
            for k in range(DKC):
                pe_t(rowblk[:, k * P:(k + 1) * P], col_tile[:, k, :])
            return gather_tiled(lambda ccin: put_tiles(ccin, rowblk[:], DKC),
                                1, DKC, F32, name)

        xr_g = gather_sym(xr_col, "xr")

        # --- Phase C1: X^2 (bf16) ---
        pc_st = ExitStack()
        pc_ = pc_st.enter_context(tc.tile_pool(name="ph_c", bufs=1))
        xt_b = pc_.tile([P, NKC, FB], BF16)
        nc.vector.tensor_copy(xt_b[:], xt[:])
        x2t_b = pc_.tile([P, NKC, FB], BF16)

        def ev_x2(m, pss):
            nc.vector.tensor_copy(x2t[:, m, :], pss[0][:])
            nc.vector.tensor_copy(x2t_b[:, m, :], pss[0][:])
        mm_pass([xt_b], NKC, NKC, ev_x2, BF16, "xslab", g=xfull_g, jpr=RJ)

        # --- R: Xr^2, Xr^3 (gathers hidden under the X^2 pass) ---
        xr2_col = pr.tile([P, DKC, FBR], F32R)
        mm_pass([xr_col], DKC, DKC,
                lambda m, pss: nc.vector.tensor_copy(xr2_col[:, m, :], pss[0][:]),
                F32R, "fslab", nb=FBR, g=xr_g, tiles=DKC)
        xr3_col = pr.tile([P, DKC, FBR], F32)
        mm_pass([xr2_col], DKC, DKC,
                lambda m, pss: nc.vector.tensor_copy(xr3_col[:, m, :], pss[0][:]),
                F32R, "fslab", nb=FBR, g=xr_g, tiles=DKC)
        xr3_g = gather_sym(xr3_col, "xr3")

        # --- Phase C2: X^3 (bf16) ---
        x3t_b = pc_.tile([P, NKC, FB], BF16)
        mm_pass([x2t_b], NKC, NKC,
                lambda m, pss: nc.vector.tensor_copy(x3t_b[:, m, :], pss[0][:]),
                BF16, "xslab", g=xfull_g, jpr=RJ)

        x3row_b = pc_.tile([P, RJ, N], BF16)
        ccin_x3 = dram.tile([RJ * NKC * P, P], BF16, name="ccin_x3")
        for j in range(RJ):
            for k in range(NKC):
                pe_t(x3row_b[:, j, k * P:(k + 1) * P],
                     x3t_b[:, k, j * P:(j + 1) * P], bf=True)
            put_tiles(ccin_x3[j * NKC * P:(j + 1) * NKC * P, :],
                      x3row_b[:, j, :], NKC)
        x3full_g = dram.tile([N_CORES * RJ * NKC * P, P], BF16,
                             addr_space="Shared", name="full_x3")
        nc.gpsimd.collective_compute(
            "AllGather", AL.bypass, replica_groups=LGROUP,
            ins=[ccin_x3.opt()], outs=[x3full_g.opt()])

        pc_st.close()

        # --- R: T_R = B1r + Y*B2r ; m1_R = B0r + Y*T_R  (xr3 gather done
        #     during the X^3 pass) ---
        b2r = pr.tile([P, DKC, FBR], F32R)
        for m in range(DKC):
            eyef = load_eye(eye_feat, m, FBR)
            combo(b2r[:, m, :], eyef, xr_col[:, m, :], xr2_col[:, m, :],
                  EC[6], EC[7], EC[8])
        tr_col = pr.tile([P, DKC, FBR], F32R)

        def ev_tr(m, pss):
            eyef = load_eye(eye_feat, m, FBR)
            b1t = scrp.tile([P, FB], F32, tag="combo", bufs=3, name="b1_scr")
            b1 = b1t[:, :FBR]
            combo(b1, eyef, xr_col[:, m, :], xr2_col[:, m, :],
                  EC[3], EC[4], EC[5])
            nc.vector.tensor_add(tr_col[:, m, :], pss[0][:], b1)
        mm_pass([b2r], DKC, DKC, ev_tr, F32R, "fslab", nb=FBR,
                g=xr3_g, tiles=DKC)

        m1r_col = pr.tile([P, DKC, FBR], F32)

        def ev_m1r(m, pss):
            eyef = load_eye(eye_feat, m, FBR)
            b0t = scrp.tile([P, FB], F32, tag="combo", bufs=3, name="b0_scr")
            b0 = b0t[:, :FBR]
            combo(b0, eyef, xr_col[:, m, :], xr2_col[:, m, :],
                  EC[0], EC[1], EC[2])
            nc.vector.tensor_add(m1r_col[:, m, :], pss[0][:], b0)
        mm_pass([tr_col], DKC, DKC, ev_m1r, F32R, "fslab", nb=FBR,
                g=xr3_g, tiles=DKC)

        m1r_g = gather_sym(m1r_col, "m1r")
        pr_st.close()

        # --- Phase D: T/S then E/P Horner steps (bf16) ---
        pd_st = ExitStack()
        pd = pd_st.enter_context(tc.tile_pool(name="ph_d", bufs=1))
        b2e_b = pd.tile([P, NKC, FB], BF16)
        c2p_b = pd.tile([P, NKC, FB], BF16)
        for m in range(NKC):
            eyet = load_eye(eye_colT, m)
            combo(b2e_b[:, m, :], eyet, xt[:, m, :], x2t[:, m, :],
                  EC[6], EC[7], EC[8])
            combo(c2p_b[:, m, :], eyet, xt[:, m, :], x2t[:, m, :],
                  PC[6], PC[7], PC[8])

        tt_b = pd.tile([P, NKC, FB], BF16)
        st_b = pd.tile([P, NKC, FB], BF16)

        def ev_ts(m, pss):
            eyet = load_eye(eye_colT, m)
            b1 = scrp.tile([P, FB], F32, tag="combo", bufs=3, name="ts_scr")
            combo(b1[:], eyet, xt[:, m, :], x2t[:, m, :], EC[3], EC[4], EC[5])
            nc.vector.tensor_add(tt_b[:, m, :], pss[0][:], b1[:])
            combo(b1[:], eyet, xt[:, m, :], x2t[:, m, :], PC[3], PC[4], PC[5])
            nc.vector.tensor_add(st_b[:, m, :], pss[1][:], b1[:])
        mm_pass([b2e_b, c2p_b], NKC, NKC, ev_ts, BF16, "xslab",
                g=x3full_g, jpr=RJ)

        def ev_ep(m, pss):
            eyet = load_eye(eye_colT, m)
            b0 = scrp.tile([P, FB], F32, tag="combo", bufs=3, name="ep_scr")
            combo(b0[:], eyet, xt[:, m, :], x2t[:, m, :], EC[0], EC[1], EC[2])
            nc.vector.tensor_add(et[:, m, :], pss[0][:], b0[:])
            combo(b0[:], eyet, xt[:, m, :], x2t[:, m, :], PC[0], PC[1], PC[2])
            nc.vector.tensor_add(m2t[:, m, :], pss[1][:], b0[:])
        mm_pass([tt_b, st_b], NKC, NKC, ev_ep, BF16, "xslab",
                g=x3full_g, jpr=RJ)

        pd_st.close()

        # --- Phase E: forcing + 9-step recurrence (fp32r) ---
        pe = top.enter_context(tc.tile_pool(name="ph_e", bufs=1))
        m1r_sb = pe.tile([P, DKC, DKC, P], F32R)
        nc.sync.dma_start(
            m1r_sb[:],
            m1r_g[:].bitcast(F32R).rearrange("(c t p) n -> p c t n",
                                             c=N_CORES, t=DKC, p=P))

        ft = pe.tile([P, DKC, FB], F32)
        mm_pass([m2t], NKC, DKC,
                lambda m, pss: nc.vector.tensor_copy(ft[:, m, :], pss[0][:]),
                F32R, "icslab0", plain=x0_full[:])

        ic_g = None
        for t in range(NSTEPS):
            # V = (m1_L @ IC)^T col block = IC^T-contract with m1_L^T col
            v = pe.tile([P, DKC, FB], F32R, tag="v_step", bufs=2, name="v")
            if t == 0:
                mm_pass([et], NKC, DKC,
                        lambda m, pss: nc.vector.tensor_copy(v[:, m, :], pss[0][:]),
                        F32R, "icslab0", plain=x_full[:])
            else:
                mm_pass([et], NKC, DKC,
                        lambda m, pss: nc.vector.tensor_copy(v[:, m, :], pss[0][:]),
                        F32R, "icslab", g=ic_g, jpr=RJ, tiles=DKC)
            # IC_new^T col = m1_R-contract with V + F^T
            icnt = pe.tile([P, DKC, FB], F32, tag="icnt_step", bufs=2, name="icnt")
            for m in range(DKC):
                ps = psum.tile([P, FB], F32, tag="mm0", bufs=2, name="ps_rec")
                for k in range(DKC):
                    nc.tensor.matmul(
                        ps[:], m1r_sb[:, k, m, :], v[:, k, :],
                        start=(k == 0), stop=(k == DKC - 1))
                nc.vector.tensor_add(icnt[:, m, :], ps[:], ft[:, m, :])
            # transpose to row block; DMA tiles out as they complete
            icrow = pe.tile([P, RJ, D], F32, tag="icrow_step", bufs=2, name="icrow")
            if t < NSTEPS - 1:
                ccin_ic = dram.tile([RJ * DKC * P, P], F32, tag="ccin_ic",
                                    name=f"ccin_ic{t}")
                for j in range(RJ):
                    for m in range(DKC):
                        pe_t(icrow[:, j, m * P:(m + 1) * P],
                             icnt[:, m, j * P:(j + 1) * P])
                        nc.sync.dma_start(
                            ccin_ic[(j * DKC + m) * P:(j * DKC + m + 1) * P, :],
                            icrow[:, j, m * P:(m + 1) * P])
                ic_g = dram.tile([N_CORES * RJ * DKC * P, P], F32,
                                 addr_space="Shared", name=f"full_ic{t}")
                nc.gpsimd.collective_compute(
                    "AllGather", AL.bypass, replica_groups=LGROUP,
                    ins=[ccin_ic.opt()], outs=[ic_g.opt()])
            else:
                for j in range(RJ):
                    for m in range(DKC):
                        pe_t(icrow[:, j, m * P:(m + 1) * P],
                             icnt[:, m, j * P:(j + 1) * P])
                    nc.sync.dma_start(z_loc[j * P:(j + 1) * P, :], icrow[:, j, :])

    nc.compile()
    return nc


_NC_CACHE = []


def _get_nc():
    if not _NC_CACHE:
        _NC_CACHE.append(build_nc())
    return _NC_CACHE[0]


def make_in_maps(inputs):
    x = np.ascontiguousarray(np.asarray(inputs["x"], dtype=np.float32))
    x0 = np.ascontiguousarray(np.asarray(inputs["x0"], dtype=np.float32))
    adj = np.ascontiguousarray(np.asarray(inputs["adj"], dtype=np.float32))
    alpha = np.ascontiguousarray(np.asarray(inputs["alpha_train"], dtype=np.float32))
    w = np.ascontiguousarray(np.asarray(inputs["w"], dtype=np.float32))
    d = np.ascontiguousarray(np.asarray(inputs["d"], dtype=np.float32))

    eye_n = np.eye(N, dtype=np.float32)
    eye_d = np.eye(D, dtype=np.float32)

    in_maps = []
    for c in range(N_CORES):
        r0 = c * RB
        f0 = c * FBR
        in_maps.append({
            "adj_rows": np.ascontiguousarray(adj[r0:r0 + RB, :]),
            "eye_rows": np.ascontiguousarray(eye_n[r0:r0 + RB, :]),
            "eye_colT": np.ascontiguousarray(eye_n[:, r0:r0 + RB]),
            "alpha_blk": np.ascontiguousarray(alpha[r0:r0 + RB]),
            "x_full": x,
            "x0_full": x0,
            "w_cols": np.ascontiguousarray(w[:, f0:f0 + FBR]),
            "w_rows": np.ascontiguousarray(w[f0:f0 + FBR, :]),
            "eye_feat": np.ascontiguousarray(eye_d[:, f0:f0 + FBR]),
            "d_full": d,
        })
    return in_maps


def kernel(**inputs) -> np.ndarray:
    nc = _get_nc()
    in_maps = make_in_maps(inputs)
    res = run_bass_kernel_spmd(nc, in_maps, core_ids=list(range(N_CORES)))
    z = np.concatenate([res.results[c]["z_loc"] for c in range(N_CORES)], axis=0)
    return np.ascontiguousarray(z.astype(np.float32))


if __name__ == "__main__":
    rng = np.random.default_rng(0)
    ins = {
        "x": rng.standard_normal((N, D)).astype(np.float32),
        "x0": rng.standard_normal((N, D)).astype(np.float32),
        "adj": (rng.random((N, N)) / N).astype(np.float32),
        "alpha_train": rng.standard_normal((N,)).astype(np.float32),
        "w": (np.eye(D) + 0.02 * rng.standard_normal((D, D))).astype(np.float32),
        "d": rng.random((D,)).astype(np.float32),
    }
    out = kernel(**ins)
    print("kernel output:", out.shape, out.dtype, float(np.linalg.norm(out)))



# revision 16
# speedup vs baseline: 1.6423x; 1.6423x over previous
"""Trainium2 Bass kernel for the ETD1 ODE block (nn_ODEblockW_28922309771809).

Math (mirrors the jax reference; identity-split, degree-4 Taylor):
  X    = dt*A = diag(0.05*sigmoid(alpha)) @ (adj - I),   ||X|| ~ 0.073
  Y    = X^2
  m1_L = I + L',  L' = X + Y/2 + Y@(X/6 + Y/24)           (deg-4 e^X)
  m2   = dt*I + P'',  P'' = dt*(X/2 + Y/6 + Y@(X/24 + Y/120))
  F    = m2@x0 = P''@x0 + dt*x0
  q    = dt * (w*clip(d,0,1)) @ w.T        (dt=0.1; symmetric)
  m1_R = e^{dt(wmat-I)} = e^{-dt} e^{q} = a*I + R'',  a = e^{-0.1}
         R'' = a*(q + Yq/2 + Yq@(q/6 + Yq/24)),  Yq = q^2  (deg-4)
  step: V = L'@IC + IC ;  IC' = a*V + V@R'' + F            (9 steps)

Deg-4 truncation ~1e-6. bf16 series matmuls + bf16 IC gathers with exact
fp32 I-terms measure 1.75e-3 frob rel err vs the reference (numpy
emulation) — far under the 2e-2 gate. The I-split means no identity is
ever materialized on device and the dominant (diagonal) part of each
operator is applied exactly in fp32.

Distribution over 8 cores: node dim sharded 256 rows/core; local tensors
held as transposed column blocks [*, 256]. Feature dim sharded 128/core
for the R-side series. Gathered tensors use 256-wide [128,256] tiles so
bf16 slab loads keep 512B contiguous runs. The two n x n gathers (X, Y)
are split into j-halves (local row chunk 0/1); consuming matmuls run j=0
first. Per-step IC gathers are split into two feature-half gathers, each
launched right after its half of the R-contract, so the next step's V
matmuls for feature chunks 0..3 only wait on the first half.
"""

import math
from contextlib import ExitStack

import numpy as np

import concourse.mybir as mybir
import concourse.tile as tile
from concourse import bacc
from concourse.bass_utils import run_bass_kernel_spmd
from concourse.masks import make_identity

F32 = mybir.dt.float32
F32R = mybir.dt.float32r
BF16 = mybir.dt.bfloat16
AL = mybir.AluOpType

N_CORES = 8
P = 128
N = 2048          # nodes
D = 1024          # features
RB = 256          # node rows per core
FB = 256          # node col-block width / wide-tile width
FBR = 128         # feature cols per core
NKC = N // P      # 16
DKC = D // P      # 8
RJ = RB // P      # 2
NSTEPS = 9        # int(1.0 // 0.1)
ALPHA_R = math.exp(-0.1)

LGROUP = [list(range(N_CORES))]


def build_nc():
    nc = bacc.Bacc("TRN2", target_bir_lowering=False, debug=False,
                   num_devices=N_CORES)

    # ---- per-core inputs (host-sliced; same NEFF on all cores) ----
    am_rows = nc.dram_tensor("am_rows", [RB, N], F32, kind="ExternalInput")
    alpha_blk = nc.dram_tensor("alpha_blk", [RB], F32, kind="ExternalInput")
    x_full = nc.dram_tensor("x_full", [N, D], F32, kind="ExternalInput")
    x0_full = nc.dram_tensor("x0_full", [N, D], F32, kind="ExternalInput")
    x_rows = nc.dram_tensor("x_rows", [RB, D], F32, kind="ExternalInput")
    x0_rows = nc.dram_tensor("x0_rows", [RB, D], F32, kind="ExternalInput")
    wT_full = nc.dram_tensor("wT_full", [D, D], F32, kind="ExternalInput")
    wTc = nc.dram_tensor("wTc", [D, FBR], F32, kind="ExternalInput")
    d_full = nc.dram_tensor("d_full", [D], F32, kind="ExternalInput")
    z_loc = nc.dram_tensor("z_loc", [RB, D], F32, kind="ExternalOutput")

    with tile.TileContext(nc) as tc, ExitStack() as top:
        const = top.enter_context(tc.tile_pool(name="const", bufs=1))
        dram = top.enter_context(tc.tile_pool(name="dram", bufs=1, space="DRAM"))
        psum = top.enter_context(tc.tile_pool(name="psum", bufs=2, space="PSUM"))
        scrp = top.enter_context(tc.tile_pool(name="scrp", bufs=1))
        lout = top.enter_context(tc.tile_pool(name="lout", bufs=1))

        ident = const.tile([P, P], F32)
        make_identity(nc, ident)
        ident_b = const.tile([P, P], BF16)
        nc.vector.tensor_copy(ident_b[:], ident[:])

        def pe_t(dst_slice, src_slice, bf=False):
            """dst[128,128] = src[128,128].T via PE transpose (identity and
            psum dtype follow the SOURCE; the copy out converts if needed)."""
            if src_slice.dtype == F32R:
                src_slice = src_slice.bitcast(F32)
            src_bf = src_slice.dtype == BF16
            ps = psum.tile([P, P], BF16 if src_bf else F32, tag="tr", bufs=2,
                           name="ps_tr")
            nc.tensor.transpose(ps[:], src_slice,
                                ident_b[:] if src_bf else ident[:])
            nc.vector.tensor_copy(dst_slice, ps[:])

        def agather(ccin, name):
            full = dram.tile([N_CORES * ccin.shape[0], ccin.shape[1]],
                             ccin.dtype, addr_space="Shared",
                             name=f"full_{name}")
            nc.gpsimd.collective_compute(
                "AllGather", AL.bypass, replica_groups=LGROUP,
                ins=[ccin.opt()], outs=[full.opt()])
            return full

        # =========================================================
        # Prep: scales
        # =========================================================
        s_sb = const.tile([P, RJ], F32)
        nc.sync.dma_start(s_sb[:], alpha_blk.ap().rearrange("(j p) -> p j", p=P))
        nc.scalar.activation(s_sb[:], s_sb[:], mybir.ActivationFunctionType.Sigmoid)
        nc.vector.tensor_scalar_mul(s_sb[:], s_sb[:], 0.05)

        dc_sb = const.tile([P, DKC], F32)
        nc.sync.dma_start(dc_sb[:], d_full.ap().rearrange("(q p) -> p q", p=P))
        nc.vector.tensor_scalar(dc_sb[:], dc_sb[:], 0.0, 1.0, AL.max, AL.min)

        # =========================================================
        # Phase A: build X row block; launch X gathers (j-halves, bf16)
        # =========================================================
        pa_st = ExitStack()
        pa = pa_st.enter_context(tc.tile_pool(name="ph_a", bufs=1))
        pr_st = ExitStack()
        pr = pr_st.enter_context(tc.tile_pool(name="ph_r", bufs=1))
        pax_st = ExitStack()
        pax = pax_st.enter_context(tc.tile_pool(name="ph_ax", bufs=1))

        xrow = pax.tile([P, RJ, N], F32)
        ccin_x = [dram.tile([(NKC // 2) * P, FB], BF16, name=f"ccin_x{j}")
                  for j in range(RJ)]
        for j in range(RJ):
            nc.sync.dma_start(xrow[:, j, :], am_rows[j * P:(j + 1) * P, :])
            nc.vector.tensor_scalar_mul(xrow[:, j, :], xrow[:, j, :],
                                        s_sb[:, j:j + 1])
            for t in range(NKC // 2):
                scb = scrp.tile([P, FB], BF16, tag="ccb", bufs=3, name="ccb")
                nc.vector.tensor_copy(scb[:], xrow[:, j, t * FB:(t + 1) * FB])
                nc.sync.dma_start(ccin_x[j][t * P:(t + 1) * P, :], scb[:])
        xg = [agather(ccin_x[j], f"x{j}") for j in range(RJ)]

        # =========================================================
        # R-side: wmat pass -> q (feature col blocks); launch q gather
        # =========================================================
        prw_st = ExitStack()
        prw = prw_st.enter_context(tc.tile_pool(name="ph_rw", bufs=1))

        wt_sb = prw.tile([P, DKC, D], BF16)      # w^T, k-chunk-major
        vrb = prw.tile([P, DKC, FBR], BF16)      # diag(dc) @ w^T[:, core cols]
        for k in range(DKC):
            wrow = prw.tile([P, D], F32, tag="w_in", bufs=2, name="wrow")
            nc.sync.dma_start(wrow[:], wT_full[k * P:(k + 1) * P, :])
            nc.vector.tensor_copy(wt_sb[:, k, :], wrow[:])
            wtc_k = scrp.tile([P, FBR], F32, tag="wtc", bufs=2, name="wtc_k")
            nc.sync.dma_start(wtc_k[:], wTc[k * P:(k + 1) * P, :])
            sc = scrp.tile([P, FBR], F32, tag="wsc", bufs=2, name="wsc")
            nc.vector.tensor_scalar_mul(sc[:], wtc_k[:], dc_sb[:, k:k + 1])
            nc.vector.tensor_copy(vrb[:, k, :], sc[:])

        q_col = pr.tile([P, DKC, FBR], F32)
        q_colb = pr.tile([P, DKC, FBR], BF16)
        for m in range(DKC):
            ps = psum.tile([P, FBR], F32, tag="mmf", bufs=2, name="ps_f")
            for k in range(DKC):
                nc.tensor.matmul(ps[:], wt_sb[:, k, m * P:(m + 1) * P],
                                 vrb[:, k, :], start=(k == 0), stop=(k == DKC - 1))
            nc.vector.tensor_scalar_mul(q_col[:, m, :], ps[:], 0.1)
            nc.vector.tensor_copy(q_colb[:, m, :], q_col[:, m, :])

        def feat_gather(colb, name):
            """Symmetric [D,D] bf16: transpose col block -> tiled ccin -> AG."""
            rowb = pr.tile([P, D], BF16, tag="f_rowb", bufs=2, name=f"rb_{name}")
            for k in range(DKC):
                pe_t(rowb[:, k * P:(k + 1) * P], colb[:, k, :], bf=True)
            ccin = dram.tile([(DKC // 2) * P, FB], BF16, name=f"ccin_{name}")
            for t in range(DKC // 2):
                nc.sync.dma_start(ccin[t * P:(t + 1) * P, :],
                                  rowb[:, t * FB:(t + 1) * FB])
            return agather(ccin, name)

        q_g = feat_gather(q_colb, "q")
        prw_st.close()

        # =========================================================
        # Phase B: xt transposes; X^2 pass; launch Y gathers
        # =========================================================
        xt = pa.tile([P, NKC, FB], F32)          # X^T col block
        xt_b = pa.tile([P, NKC, FB], BF16)
        for k in range(NKC):
            for j in range(RJ):
                pe_t(xt[:, k, j * P:(j + 1) * P], xrow[:, j, k * P:(k + 1) * P])
        nc.vector.tensor_copy(xt_b[:], xt[:])
        pax_st.close()
        slabn_st = ExitStack()
        slabn = slabn_st.enter_context(tc.tile_pool(name="slab_n", bufs=1))

        x2t = pa.tile([P, NKC, FB], F32)

        def nslab_load(g2, mp, tag):
            """[128, 8ranks, 2j, 256] bf16 slab for output m-pair mp from the
            two j-half gathers of an n x n matrix."""
            sl = slabn.tile([P, N_CORES, RJ, FB], BF16, tag=tag, bufs=2,
                            name=f"slab_{tag}")
            for j in range(RJ):
                a = g2[j][:].rearrange("(c t p) n -> p c t n", c=N_CORES, p=P)
                nc.sync.dma_start(sl[:, :, j, :], a[:, :, mp, :])
            return sl

        def npass(g2, rhs_list, evict, tag):
            """out[m] = sum_k Full[k,m].T @ rhs[i][k] for 16 m-chunks; Full from
            j-half gathers g2; j=0 contributions run first."""
            for mp in range(NKC // 2):
                sl = nslab_load(g2, mp, tag)
                for mh in range(2):
                    m = mp * 2 + mh
                    pss = [psum.tile([P, FB], F32, tag=f"mm{i}", bufs=2,
                                     name=f"ps_mm{i}")
                           for i in range(len(rhs_list))]
                    nk = 0
                    for j in range(RJ):
                        for c in range(N_CORES):
                            lt = sl[:, c, j, mh * P:(mh + 1) * P]
                            for ps, rhs in zip(pss, rhs_list):
                                nc.tensor.matmul(ps[:], lt, rhs[:, c * RJ + j, :],
                                                 start=(nk == 0),
                                                 stop=(nk == NKC - 1))
                            nk += 1
                    evict(m, pss)

        def ev_x2(m, pss):
            nc.vector.tensor_copy(x2t[:, m, :], pss[0][:])
        npass(xg, [xt_b], ev_x2, "xslab")

        # Y gathers (j-halves): fp32 transpose of x2t with bf16 convert-copy
        pay_st = ExitStack()
        pay = pay_st.enter_context(tc.tile_pool(name="ph_ay", bufs=1))
        yrow_b = pay.tile([P, RJ, N], BF16)
        ccin_y = [dram.tile([(NKC // 2) * P, FB], BF16, name=f"ccin_y{j}")
                  for j in range(RJ)]
        for j in range(RJ):
            for k in range(NKC):
                pe_t(yrow_b[:, j, k * P:(k + 1) * P],
                     x2t[:, k, j * P:(j + 1) * P])
            for t in range(NKC // 2):
                nc.sync.dma_start(ccin_y[j][t * P:(t + 1) * P, :],
                                  yrow_b[:, j, t * FB:(t + 1) * FB])
        yg = [agather(ccin_y[j], f"y{j}") for j in range(RJ)]
        pay_st.close()
        slabf_st = ExitStack()
        slabf = slabf_st.enter_context(tc.tile_pool(name="slab_f", bufs=1))

        # =========================================================
        # R-side: Yq pass (q_g), R'' pass (yq_g), r2 gather
        # =========================================================
        def fslab_load(g, mp, tag):
            sl = slabf.tile([P, DKC, FB], BF16, tag=tag, bufs=2,
                            name=f"slab_{tag}")
            a = g[:].rearrange("(c t p) n -> p c t n", c=N_CORES, p=P)
            nc.sync.dma_start(sl[:], a[:, :, mp, :])
            return sl

        def fpass(g, rhs, evict, tag):
            for mp in range(DKC // 2):
                sl = fslab_load(g, mp, tag)
                for mh in range(2):
                    m = mp * 2 + mh
                    ps = psum.tile([P, FBR], F32, tag="mmf", bufs=2, name="ps_f")
                    for k in range(DKC):
                        nc.tensor.matmul(ps[:], sl[:, k, mh * P:(mh + 1) * P],
                                         rhs[:, k, :], start=(k == 0),
                                         stop=(k == DKC - 1))
                    evict(m, ps)

        yq_col = pr.tile([P, DKC, FBR], F32)
        b1q_b = pr.tile([P, DKC, FBR], BF16)

        def ev_yq(m, ps):
            nc.vector.tensor_copy(yq_col[:, m, :], ps[:])
            sc = scrp.tile([P, FBR], F32, tag="fco", bufs=2, name="fco")
            nc.vector.tensor_scalar_mul(sc[:], ps[:], 1.0 / 24.0)
            nc.vector.scalar_tensor_tensor(sc[:], q_col[:, m, :], 1.0 / 6.0,
                                           sc[:], AL.mult, AL.add)
            nc.vector.tensor_copy(b1q_b[:, m, :], sc[:])
        fpass(q_g, q_colb, ev_yq, "fslab")

        yq_colb = pr.tile([P, DKC, FBR], BF16)
        nc.vector.tensor_copy(yq_colb[:], yq_col[:])
        yq_g = feat_gather(yq_colb, "yq")

        r2_colb = pr.tile([P, DKC, FBR], BF16)

        def ev_r2(m, ps):
            sc = scrp.tile([P, FBR], F32, tag="fco", bufs=2, name="fco")
            nc.vector.tensor_scalar_mul(sc[:], yq_col[:, m, :], 0.5 * ALPHA_R)
            nc.vector.scalar_tensor_tensor(sc[:], q_col[:, m, :], ALPHA_R,
                                           sc[:], AL.mult, AL.add)
            nc.vector.scalar_tensor_tensor(sc[:], ps[:], ALPHA_R, sc[:],
                                           AL.mult, AL.add)
            nc.vector.tensor_copy(r2_colb[:, m, :], sc[:])
        fpass(yq_g, b1q_b, ev_r2, "fslab")
        r2_g = feat_gather(r2_colb, "r2")
        slabf_st.close()

        # =========================================================
        # Phase C: E/P pass -> l2t (L'^T col), p2t (P''^T col)
        # =========================================================
        b1e_b = pa.tile([P, NKC, FB], BF16)
        b1p_b = pa.tile([P, NKC, FB], BF16)
        for m in range(NKC):
            sc = scrp.tile([P, FB], F32, tag="nco", bufs=3, name="nco")
            nc.vector.tensor_scalar_mul(sc[:], x2t[:, m, :], 1.0 / 24.0)
            nc.vector.scalar_tensor_tensor(sc[:], xt[:, m, :], 1.0 / 6.0,
                                           sc[:], AL.mult, AL.add)
            nc.vector.tensor_copy(b1e_b[:, m, :], sc[:])
            nc.vector.tensor_scalar_mul(sc[:], x2t[:, m, :], 1.0 / 120.0)
            nc.vector.scalar_tensor_tensor(sc[:], xt[:, m, :], 1.0 / 24.0,
                                           sc[:], AL.mult, AL.add)
            nc.vector.tensor_copy(b1p_b[:, m, :], sc[:])

        l2t = lout.tile([P, NKC, FB], F32R)      # L'^T col block
        l2tb = lout.tile([P, NKC, FB], BF16)
        p2t = pa.tile([P, NKC, FB], F32R)        # P''^T col block

        def ev_ep(m, pss):
            sc = scrp.tile([P, FB], F32, tag="nco", bufs=3, name="nco")
            nc.vector.tensor_scalar_mul(sc[:], x2t[:, m, :], 0.5)
            nc.vector.scalar_tensor_tensor(sc[:], xt[:, m, :], 1.0, sc[:],
                                           AL.mult, AL.add)
            nc.vector.tensor_add(l2t[:, m, :], pss[0][:], sc[:])
            nc.vector.tensor_copy(l2tb[:, m, :], l2t[:, m, :].bitcast(F32))
            nc.vector.tensor_scalar_mul(sc[:], x2t[:, m, :], 1.0 / 60.0)
            nc.vector.scalar_tensor_tensor(sc[:], xt[:, m, :], 1.0 / 20.0,
                                           sc[:], AL.mult, AL.add)
            nc.vector.scalar_tensor_tensor(p2t[:, m, :], pss[1][:], 0.1,
                                           sc[:], AL.mult, AL.add)
        npass(yg, [b1e_b, b1p_b], ev_ep, "xslab")
        slabn_st.close()

        # =========================================================
        # Forcing: F^T col = P''-contract(x0)  (+0.1*x0^T added below)
        # =========================================================
        slabp_st = ExitStack()
        slabp = slabp_st.enter_context(tc.tile_pool(name="slab_p", bufs=1))
        ft = lout.tile([P, DKC, FB], F32)

        def plain_pass(plain, rhs, evict, tag):
            """lhsT k-chunks from a [N, D] fp32 DRAM tensor; fp32r matmuls."""
            for m in range(DKC):
                sl = slabp.tile([P, NKC, P], F32R, tag=tag, bufs=2,
                                name=f"slab_{tag}")
                nc.sync.dma_start(
                    sl[:], plain[:, m * P:(m + 1) * P].bitcast(F32R).rearrange(
                        "(k p) n -> p k n", p=P))
                ps = psum.tile([P, FB], F32, tag="mm0", bufs=2, name="ps_mm0")
                for k in range(NKC):
                    nc.tensor.matmul(ps[:], sl[:, k, :], rhs[:, k, :],
                                     start=(k == 0), stop=(k == NKC - 1))
                evict(m, ps)

        def ev_ft(m, ps):
            nc.vector.tensor_copy(ft[:, m, :], ps[:])
        plain_pass(x0_full, p2t, ev_ft, "icslab0")

        # x/x0 col blocks (exact I-terms), step-0-only pool
        pf_st = ExitStack()
        pf = pf_st.enter_context(tc.tile_pool(name="ph_f", bufs=1))
        x0colT = pf.tile([P, DKC, FB], F32)
        xcolT = pf.tile([P, DKC, FB], F32)
        for src, dst in ((x0_rows, x0colT), (x_rows, xcolT)):
            for j in range(RJ):
                rsb = pf.tile([P, D], F32, tag="rows_in", bufs=2, name="rows_in")
                nc.sync.dma_start(rsb[:], src[j * P:(j + 1) * P, :])
                for m in range(DKC):
                    pe_t(dst[:, m, j * P:(j + 1) * P],
                         rsb[:, m * P:(m + 1) * P])
        for m in range(DKC):
            nc.vector.scalar_tensor_tensor(ft[:, m, :], x0colT[:, m, :], 0.1,
                                           ft[:, m, :], AL.mult, AL.add)

        # =========================================================
        # Recurrence: 9 steps
        # =========================================================
        GCH = 2                   # feature-half gathers per step
        MG = DKC // GCH           # m-chunks per gather chunk (4)

        def r_contract_and_ship(t, v, v_b):
            """R-contract + evict + transpose + chunked gathers (or output)."""
            icnt = pe.tile([P, DKC, FB], F32, tag="icnt", bufs=2, name="icnt")
            icrow_b = pe.tile([P, RJ, D], BF16, tag="icrow", bufs=2,
                              name="icrow_b")
            ic_g = []
            for g in range(GCH):
                for mh in range(MG):
                    m = g * MG + mh
                    ps = psum.tile([P, FB], F32, tag="mm1", bufs=2, name="ps_r")
                    for k in range(DKC):
                        nc.tensor.matmul(
                            ps[:],
                            r2_sb[:, k, m // 2, (m % 2) * P:(m % 2 + 1) * P],
                            v_b[:, k, :], start=(k == 0), stop=(k == DKC - 1))
                    nc.vector.scalar_tensor_tensor(icnt[:, m, :], v[:, m, :],
                                                   ALPHA_R, ps[:], AL.mult,
                                                   AL.add)
                    nc.vector.tensor_add(icnt[:, m, :], icnt[:, m, :],
                                         ft[:, m, :])
                if t == NSTEPS - 1:
                    continue
                # transpose bf16, ship as [128,256] tiles: (j, mp) pairs
                ccin = dram.tile([RJ * (MG // 2) * P, FB], BF16,
                                 tag=f"ccin_ic{g}", bufs=2,
                                 name=f"ccin_ic{t}_{g}")
                for j in range(RJ):
                    for mh in range(MG):
                        m = g * MG + mh
                        pe_t(icrow_b[:, j, m * P:(m + 1) * P],
                             icnt[:, m, j * P:(j + 1) * P], bf=True)
                    for mp in range(MG // 2):
                        f0 = (g * MG + mp * 2) * P
                        nc.sync.dma_start(
                            ccin[(j * (MG // 2) + mp) * P:
                                 (j * (MG // 2) + mp + 1) * P, :],
                            icrow_b[:, j, f0:f0 + FB])
                ic_g.append(agather(ccin, f"ic{t}_{g}"))
            if t == NSTEPS - 1:
                icrow_f = pe.tile([P, RJ, D], F32, name="icrow_f")
                for j in range(RJ):
                    for m in range(DKC):
                        pe_t(icrow_f[:, j, m * P:(m + 1) * P],
                             icnt[:, m, j * P:(j + 1) * P])
                    nc.sync.dma_start(z_loc[j * P:(j + 1) * P, :],
                                      icrow_f[:, j, :])
            return icnt, ic_g

        # --- step 0: V from fp32 x directly (no quantization) ---
        v = lout.tile([P, DKC, FB], F32, tag="v", bufs=2, name="v")
        v_b = lout.tile([P, DKC, FB], BF16, tag="vb", bufs=2, name="v_b")

        def ev_v0(m, ps):
            nc.vector.tensor_add(v[:, m, :], ps[:], xcolT[:, m, :])
            nc.vector.tensor_copy(v_b[:, m, :], v[:, m, :])
        plain_pass(x_full, l2t, ev_v0, "icslab0")
        pf_st.close()
        slabp_st.close()
        pr_st.close()
        pa_st.close()
        pe = top.enter_context(tc.tile_pool(name="ph_e", bufs=1))
        slabic = top.enter_context(tc.tile_pool(name="slab_ic", bufs=1))
        r2_sb = pe.tile([P, DKC, DKC // 2, FB], BF16)
        nc.sync.dma_start(
            r2_sb[:], r2_g[:].rearrange("(c t p) n -> p c t n",
                                        c=N_CORES, p=P))
        icnt_prev, ic_g = r_contract_and_ship(0, v, v_b)

        # --- steps 1..8 ---
        for t in range(1, NSTEPS):
            v = lout.tile([P, DKC, FB], F32, tag="v", bufs=2, name="v")
            v_b = lout.tile([P, DKC, FB], BF16, tag="vb", bufs=2, name="v_b")
            for g in range(GCH):
                ga = ic_g[g][:].rearrange("(c j t2 p) n -> p c j t2 n",
                                          c=N_CORES, j=RJ, t2=MG // 2, p=P)
                for mp in range(MG // 2):
                    sl = slabic.tile([P, N_CORES, RJ, FB], BF16, tag="icslab",
                                    bufs=2, name="slab_ic")
                    nc.sync.dma_start(sl[:], ga[:, :, :, mp, :])
                    for mh2 in range(2):
                        m = g * MG + mp * 2 + mh2
                        ps = psum.tile([P, FB], F32, tag="mm0", bufs=2,
                                       name="ps_mm0")
                        nk = 0
                        for c in range(N_CORES):
                            for j in range(RJ):
                                nc.tensor.matmul(
                                    ps[:], sl[:, c, j, mh2 * P:(mh2 + 1) * P],
                                    l2tb[:, c * RJ + j, :],
                                    start=(nk == 0), stop=(nk == NKC - 1))
                                nk += 1
                        nc.vector.tensor_add(v[:, m, :], ps[:],
                                             icnt_prev[:, m, :])
                        nc.vector.tensor_copy(v_b[:, m, :], v[:, m, :])
            icnt_prev, ic_g = r_contract_and_ship(t, v, v_b)

    nc.compile()
    return nc


_NC_CACHE = []


def _get_nc():
    if not _NC_CACHE:
        _NC_CACHE.append(build_nc())
    return _NC_CACHE[0]


def make_in_maps(inputs):
    x = np.ascontiguousarray(np.asarray(inputs["x"], dtype=np.float32))
    x0 = np.ascontiguousarray(np.asarray(inputs["x0"], dtype=np.float32))
    adj = np.asarray(inputs["adj"], dtype=np.float32)
    alpha = np.ascontiguousarray(np.asarray(inputs["alpha_train"],
                                            dtype=np.float32))
    w = np.asarray(inputs["w"], dtype=np.float32)
    d = np.ascontiguousarray(np.asarray(inputs["d"], dtype=np.float32))

    am = adj - np.eye(N, dtype=np.float32)
    wT = np.ascontiguousarray(w.T)

    in_maps = []
    for c in range(N_CORES):
        r0 = c * RB
        f0 = c * FBR
        in_maps.append({
            "am_rows": np.ascontiguousarray(am[r0:r0 + RB, :]),
            "alpha_blk": np.ascontiguousarray(alpha[r0:r0 + RB]),
            "x_full": x,
            "x0_full": x0,
            "x_rows": np.ascontiguousarray(x[r0:r0 + RB, :]),
            "x0_rows": np.ascontiguousarray(x0[r0:r0 + RB, :]),
            "wT_full": wT,
            "wTc": np.ascontiguousarray(wT[:, f0:f0 + FBR]),
            "d_full": d,
        })
    return in_maps


def kernel(**inputs) -> np.ndarray:
    nc = _get_nc()
    in_maps = make_in_maps(inputs)
    res = run_bass_kernel_spmd(nc, in_maps, core_ids=list(range(N_CORES)))
    z = np.concatenate([res.results[c]["z_loc"] for c in range(N_CORES)], axis=0)
    return np.ascontiguousarray(z.astype(np.float32))


if __name__ == "__main__":
    rng = np.random.default_rng(0)
    ins = {
        "x": rng.standard_normal((N, D)).astype(np.float32),
        "x0": rng.standard_normal((N, D)).astype(np.float32),
        "adj": (rng.random((N, N)) / N).astype(np.float32),
        "alpha_train": rng.standard_normal((N,)).astype(np.float32),
        "w": (np.eye(D) + 0.02 * rng.standard_normal((D, D))).astype(np.float32),
        "d": rng.random((D,)).astype(np.float32),
    }
    out = kernel(**ins)
    print("kernel output:", out.shape, out.dtype, float(np.linalg.norm(out)))


# revision 19
# speedup vs baseline: 1.9842x; 1.2082x over previous
"""Trainium2 Bass kernel for the ETD1 ODE block (nn_ODEblockW_28922309771809).

Math (identity-split, degree-4 Taylor, step-doubling):
  X    = dt*A = diag(0.05*sigmoid(alpha)) @ (adj - I),   ||X|| ~ 0.073
  Y    = X^2;  m1_L = I + L',  L' = X + Y/2 + Y@(X/6 + Y/24)
  m2   = dt*I + P'',  P'' = dt*(X/2 + Y/6 + Y@(X/24 + Y/120))
  F    = m2@x0 = P''@x0 + dt*x0
  q    = dt*(w*clip(d,0,1))@w.T  (symmetric);  m1_R = e^{dt(wmat-I)}
       = e^{-dt} e^{q} = a*I + R'',  R'' = a*(q + q^2/2 + q^3/6 + q^4/24)
  step:    V = L'@IC + IC ;  IC' = a*V + V@R'' + F
  Phi^2:   M2 = I + L4 (L4 = 2L' + L'^2),  R2 = a^2 I + R4 (R4 = 2a R'' + R''^2)
           F2 = Phi(F) = a*(F + L'@F) + (F + L'@F)@R'' + F
  z = Phi2(Phi2(Phi2(Phi2(Phi(x)))))       (9 steps = 1 single + 4 doubles)

Numerics (numpy emulation vs fp64 reference): 2.1e-3 frob rel err vs the
2e-2 gate (10x margin). All identity terms are applied exactly in fp32
from local column blocks; everything gathered travels in bf16.

Why doubling: 8-rank AllGathers have a ~17-30 us latency floor nearly
independent of size, and each recurrence step must gather the new state.
Doubling halves the number of chain gathers (8 -> 4); the Phi^2 operator
precompute (M2 pass, R''^2 pass, F2) is emitted between the single step's
gather and the first double step, keeping the PE busy (and its HAM clock
warm) while the gather is in flight.
"""

import math
from contextlib import ExitStack

import numpy as np

import concourse.mybir as mybir
import concourse.tile as tile
from concourse import bacc
from concourse.bass_utils import run_bass_kernel_spmd
from concourse.masks import make_identity

F32 = mybir.dt.float32
F32R = mybir.dt.float32r
BF16 = mybir.dt.bfloat16
AL = mybir.AluOpType

N_CORES = 8
P = 128
N = 2048          # nodes
D = 1024          # features
RB = 256          # node rows per core
FB = 256          # wide-tile width
FBR = 128         # feature cols per core
NKC = N // P      # 16
DKC = D // P      # 8
RJ = RB // P      # 2
ABR = math.exp(-0.1)

LGROUP = [list(range(N_CORES))]


def build_nc():
    nc = bacc.Bacc("TRN2", target_bir_lowering=False, debug=False,
                   num_devices=N_CORES)

    am_rows = nc.dram_tensor("am_rows", [RB, N], F32, kind="ExternalInput")
    alpha_blk = nc.dram_tensor("alpha_blk", [RB], F32, kind="ExternalInput")
    x_full = nc.dram_tensor("x_full", [N, D], F32, kind="ExternalInput")
    x0_full = nc.dram_tensor("x0_full", [N, D], F32, kind="ExternalInput")
    x_rows = nc.dram_tensor("x_rows", [RB, D], F32, kind="ExternalInput")
    x0_rows = nc.dram_tensor("x0_rows", [RB, D], F32, kind="ExternalInput")
    wT_full = nc.dram_tensor("wT_full", [D, D], F32, kind="ExternalInput")
    wTc = nc.dram_tensor("wTc", [D, FBR], F32, kind="ExternalInput")
    d_full = nc.dram_tensor("d_full", [D], F32, kind="ExternalInput")
    z_loc = nc.dram_tensor("z_loc", [RB, D], F32, kind="ExternalOutput")

    with tile.TileContext(nc) as tc, ExitStack() as top:
        const = top.enter_context(tc.tile_pool(name="const", bufs=1))
        dram = top.enter_context(tc.tile_pool(name="dram", bufs=1, space="DRAM"))
        psum = top.enter_context(tc.tile_pool(name="psum", bufs=2, space="PSUM"))
        scrp = top.enter_context(tc.tile_pool(name="scrp", bufs=1))
        lout = top.enter_context(tc.tile_pool(name="lout", bufs=1))

        ident = const.tile([P, P], F32)
        make_identity(nc, ident)
        ident_b = const.tile([P, P], BF16)
        nc.vector.tensor_copy(ident_b[:], ident[:])

        def pe_t(dst_slice, src_slice):
            """dst[128,128] = src[128,128].T via PE transpose; the copy-out
            converts dtype if dst differs from src."""
            if src_slice.dtype == F32R:
                src_slice = src_slice.bitcast(F32)
            src_bf = src_slice.dtype == BF16
            ps = psum.tile([P, P], BF16 if src_bf else F32, tag="tr", bufs=2,
                           name="ps_tr")
            nc.tensor.transpose(ps[:], src_slice,
                                ident_b[:] if src_bf else ident[:])
            nc.vector.tensor_copy(dst_slice, ps[:])

        def agather(ccin, name):
            full = dram.tile([N_CORES * ccin.shape[0], ccin.shape[1]],
                             ccin.dtype, addr_space="Shared",
                             name=f"full_{name}")
            nc.gpsimd.collective_compute(
                "AllGather", AL.bypass, replica_groups=LGROUP,
                ins=[ccin.opt()], outs=[full.opt()])
            return full

        # ---- scales ----
        s_sb = const.tile([P, RJ], F32)
        nc.sync.dma_start(s_sb[:], alpha_blk.ap().rearrange("(j p) -> p j", p=P))
        nc.scalar.activation(s_sb[:], s_sb[:], mybir.ActivationFunctionType.Sigmoid)
        nc.vector.tensor_scalar_mul(s_sb[:], s_sb[:], 0.05)

        dc_sb = const.tile([P, DKC], F32)
        nc.sync.dma_start(dc_sb[:], d_full.ap().rearrange("(q p) -> p q", p=P))
        nc.vector.tensor_scalar(dc_sb[:], dc_sb[:], 0.0, 1.0, AL.max, AL.min)

        # =========================================================
        # Phase A: X row block -> X gathers (j-halves, bf16)
        # =========================================================
        pa_st = ExitStack()
        pa = pa_st.enter_context(tc.tile_pool(name="ph_a", bufs=1))
        pr_st = ExitStack()
        pr = pr_st.enter_context(tc.tile_pool(name="ph_r", bufs=1))
        pax_st = ExitStack()
        pax = pax_st.enter_context(tc.tile_pool(name="ph_ax", bufs=1))

        xrow = pax.tile([P, RJ, N], F32)
        ccin_x = [dram.tile([(NKC // 2) * P, FB], BF16, name=f"ccin_x{j}")
                  for j in range(RJ)]
        for j in range(RJ):
            nc.sync.dma_start(xrow[:, j, :], am_rows[j * P:(j + 1) * P, :])
            nc.vector.tensor_scalar_mul(xrow[:, j, :], xrow[:, j, :],
                                        s_sb[:, j:j + 1])
            for t in range(NKC // 2):
                scb = scrp.tile([P, FB], BF16, tag="ccb", bufs=3, name="ccb")
                nc.vector.tensor_copy(scb[:], xrow[:, j, t * FB:(t + 1) * FB])
                nc.sync.dma_start(ccin_x[j][t * P:(t + 1) * P, :], scb[:])
        xg = [agather(ccin_x[j], f"x{j}") for j in range(RJ)]

        # =========================================================
        # R-side: wmat pass -> q; q gather
        # =========================================================
        prw_st = ExitStack()
        prw = prw_st.enter_context(tc.tile_pool(name="ph_rw", bufs=1))

        wt_sb = prw.tile([P, DKC, D], BF16)
        vrb = prw.tile([P, DKC, FBR], BF16)
        for k in range(DKC):
            wrow = prw.tile([P, D], F32, tag="w_in", bufs=2, name="wrow")
            nc.sync.dma_start(wrow[:], wT_full[k * P:(k + 1) * P, :])
            nc.vector.tensor_copy(wt_sb[:, k, :], wrow[:])
            wtc_k = scrp.tile([P, FBR], F32, tag="wtc", bufs=2, name="wtc_k")
            nc.sync.dma_start(wtc_k[:], wTc[k * P:(k + 1) * P, :])
            sc = scrp.tile([P, FBR], F32, tag="wsc", bufs=2, name="wsc")
            nc.vector.tensor_scalar_mul(sc[:], wtc_k[:], dc_sb[:, k:k + 1])
            nc.vector.tensor_copy(vrb[:, k, :], sc[:])

        q_col = pr.tile([P, DKC, FBR], F32)
        q_colb = pr.tile([P, DKC, FBR], BF16)
        for m in range(DKC):
            ps = psum.tile([P, FBR], F32, tag="mmf", bufs=2, name="ps_f")
            for k in range(DKC):
                nc.tensor.matmul(ps[:], wt_sb[:, k, m * P:(m + 1) * P],
                                 vrb[:, k, :], start=(k == 0), stop=(k == DKC - 1))
            nc.vector.tensor_scalar_mul(q_col[:, m, :], ps[:], 0.1)
            nc.vector.tensor_copy(q_colb[:, m, :], q_col[:, m, :])

        def feat_gather(colb, name):
            rowb = pr.tile([P, D], BF16, tag="f_rowb", bufs=2, name=f"rb_{name}")
            for k in range(DKC):
                pe_t(rowb[:, k * P:(k + 1) * P], colb[:, k, :])
            ccin = dram.tile([(DKC // 2) * P, FB], BF16, name=f"ccin_{name}")
            for t in range(DKC // 2):
                nc.sync.dma_start(ccin[t * P:(t + 1) * P, :],
                                  rowb[:, t * FB:(t + 1) * FB])
            return agather(ccin, name)

        q_g = feat_gather(q_colb, "q")
        prw_st.close()

        # =========================================================
        # Phase B: xt; X^2 pass; Y gathers
        # =========================================================
        xt = pa.tile([P, NKC, FB], F32)
        for k in range(NKC):
            for j in range(RJ):
                pe_t(xt[:, k, j * P:(j + 1) * P], xrow[:, j, k * P:(k + 1) * P])
        pax_st.close()
        slabn_st = ExitStack()
        slabn = slabn_st.enter_context(tc.tile_pool(name="slab_n", bufs=1))
        paxb_st = ExitStack()
        paxb = paxb_st.enter_context(tc.tile_pool(name="ph_axb", bufs=1))
        xt_b = paxb.tile([P, NKC, FB], BF16)
        nc.vector.tensor_copy(xt_b[:], xt[:])

        x2t_b = pa.tile([P, NKC, FB], BF16)

        def nslab_load2(g2, mp, tag):
            """[128, 8ranks, 2j, 256] slab for m-pair mp from two j-half
            gathers of an n x n matrix."""
            sl = slabn.tile([P, N_CORES, RJ, FB], BF16, tag=tag, bufs=2,
                            name=f"slab_{tag}")
            for j in range(RJ):
                a = g2[j][:].rearrange("(c t p) n -> p c t n", c=N_CORES, p=P)
                nc.sync.dma_start(sl[:, :, j, :], a[:, :, mp, :])
            return sl

        def npass(g2, rhs_list, evict, tag):
            for mp in range(NKC // 2):
                sl = nslab_load2(g2, mp, tag)
                for mh in range(2):
                    m = mp * 2 + mh
                    pss = [psum.tile([P, FB], F32, tag=f"mm{i}", bufs=2,
                                     name=f"ps_mm{i}")
                           for i in range(len(rhs_list))]
                    nk = 0
                    for j in range(RJ):
                        for c in range(N_CORES):
                            lt = sl[:, c, j, mh * P:(mh + 1) * P]
                            for ps, rhs in zip(pss, rhs_list):
                                nc.tensor.matmul(ps[:], lt, rhs[:, c * RJ + j, :],
                                                 start=(nk == 0),
                                                 stop=(nk == NKC - 1))
                            nk += 1
                    evict(m, pss)

        def ev_x2(m, pss):
            nc.vector.tensor_copy(x2t_b[:, m, :], pss[0][:])
        npass(xg, [xt_b], ev_x2, "xslab")
        paxb_st.close()

        pay_st = ExitStack()
        pay = pay_st.enter_context(tc.tile_pool(name="ph_ay", bufs=1))
        yrow_b = pay.tile([P, RJ, N], BF16)
        ccin_y = [dram.tile([(NKC // 2) * P, FB], BF16, name=f"ccin_y{j}")
                  for j in range(RJ)]
        for j in range(RJ):
            for k in range(NKC):
                pe_t(yrow_b[:, j, k * P:(k + 1) * P],
                     x2t_b[:, k, j * P:(j + 1) * P])
            for t in range(NKC // 2):
                nc.sync.dma_start(ccin_y[j][t * P:(t + 1) * P, :],
                                  yrow_b[:, j, t * FB:(t + 1) * FB])
        yg = [agather(ccin_y[j], f"y{j}") for j in range(RJ)]
        pay_st.close()

        # =========================================================
        # R-side Horner in q: Yq, q^3, q^4 passes (all off q_g); r2 gather
        # =========================================================
        slabf_st = ExitStack()
        slabf = slabf_st.enter_context(tc.tile_pool(name="slab_f", bufs=1))

        def fpass(g, rhs, evict, tag):
            for mp in range(DKC // 2):
                sl = slabf.tile([P, DKC, FB], BF16, tag=tag, bufs=2,
                                name=f"slab_{tag}")
                a = g[:].rearrange("(c t p) n -> p c t n", c=N_CORES, p=P)
                nc.sync.dma_start(sl[:], a[:, :, mp, :])
                for mh in range(2):
                    m = mp * 2 + mh
                    ps = psum.tile([P, FBR], F32, tag="mmf", bufs=2, name="ps_f")
                    for k in range(DKC):
                        nc.tensor.matmul(ps[:], sl[:, k, mh * P:(mh + 1) * P],
                                         rhs[:, k, :], start=(k == 0),
                                         stop=(k == DKC - 1))
                    evict(m, ps)

        yq_col = pr.tile([P, DKC, FBR], F32)
        yq_colb = pr.tile([P, DKC, FBR], BF16)

        def ev_yq(m, ps):
            nc.vector.tensor_copy(yq_col[:, m, :], ps[:])
            nc.vector.tensor_copy(yq_colb[:, m, :], ps[:])
        fpass(q_g, q_colb, ev_yq, "fslab")

        q3_col = pr.tile([P, DKC, FBR], F32)
        q3_colb = pr.tile([P, DKC, FBR], BF16)

        def ev_q3(m, ps):
            nc.vector.tensor_copy(q3_col[:, m, :], ps[:])
            nc.vector.tensor_copy(q3_colb[:, m, :], ps[:])
        fpass(q_g, yq_colb, ev_q3, "fslab")

        r2_colb = pr.tile([P, DKC, FBR], BF16)

        def ev_r2(m, ps):
            sc = scrp.tile([P, FBR], F32, tag="fco", bufs=2, name="fco")
            nc.vector.tensor_scalar_mul(sc[:], ps[:], ABR / 24.0)
            nc.vector.scalar_tensor_tensor(sc[:], q3_col[:, m, :], ABR / 6.0,
                                           sc[:], AL.mult, AL.add)
            nc.vector.scalar_tensor_tensor(sc[:], yq_col[:, m, :], ABR / 2.0,
                                           sc[:], AL.mult, AL.add)
            nc.vector.scalar_tensor_tensor(sc[:], q_col[:, m, :], ABR,
                                           sc[:], AL.mult, AL.add)
            nc.vector.tensor_copy(r2_colb[:, m, :], sc[:])
        fpass(q_g, q3_colb, ev_r2, "fslab")
        r2_g = feat_gather(r2_colb, "r2")
        slabf_st.close()

        # =========================================================
        # E/P pass -> l2t (L'^T col, F32R), p2t (P''^T col, F32R)
        # =========================================================
        b1e_b = pa.tile([P, NKC, FB], BF16)
        b1p_b = pa.tile([P, NKC, FB], BF16)
        for m in range(NKC):
            yf = scrp.tile([P, FB], F32, tag="yf", bufs=2, name="yf")
            nc.vector.tensor_copy(yf[:], x2t_b[:, m, :])
            sc = scrp.tile([P, FB], F32, tag="nco", bufs=3, name="nco")
            nc.vector.tensor_scalar_mul(sc[:], yf[:], 1.0 / 24.0)
            nc.vector.scalar_tensor_tensor(sc[:], xt[:, m, :], 1.0 / 6.0,
                                           sc[:], AL.mult, AL.add)
            nc.vector.tensor_copy(b1e_b[:, m, :], sc[:])
            nc.vector.tensor_scalar_mul(sc[:], yf[:], 1.0 / 120.0)
            nc.vector.scalar_tensor_tensor(sc[:], xt[:, m, :], 1.0 / 24.0,
                                           sc[:], AL.mult, AL.add)
            nc.vector.tensor_copy(b1p_b[:, m, :], sc[:])

        l2t = lout.tile([P, NKC, FB], F32R)
        l2tb = lout.tile([P, NKC, FB], BF16)
        p2t = pa.tile([P, NKC, FB], F32R)

        def ev_ep(m, pss):
            yf = scrp.tile([P, FB], F32, tag="yf", bufs=2, name="yf")
            nc.vector.tensor_copy(yf[:], x2t_b[:, m, :])
            sc = scrp.tile([P, FB], F32, tag="nco", bufs=3, name="nco")
            nc.vector.tensor_scalar_mul(sc[:], yf[:], 0.5)
            nc.vector.scalar_tensor_tensor(sc[:], xt[:, m, :], 1.0, sc[:],
                                           AL.mult, AL.add)
            nc.vector.tensor_add(l2t[:, m, :], pss[0][:], sc[:])
            nc.vector.tensor_copy(l2tb[:, m, :], l2t[:, m, :].bitcast(F32))
            nc.vector.tensor_scalar_mul(sc[:], yf[:], 1.0 / 60.0)
            nc.vector.scalar_tensor_tensor(sc[:], xt[:, m, :], 1.0 / 20.0,
                                           sc[:], AL.mult, AL.add)
            nc.vector.scalar_tensor_tensor(p2t[:, m, :], pss[1][:], 0.1,
                                           sc[:], AL.mult, AL.add)
        npass(yg, [b1e_b, b1p_b], ev_ep, "xslab")
        slabn_st.close()

        # L' gather (single, bf16): row block via transposes of l2tb
        plg_st = ExitStack()
        plg = plg_st.enter_context(tc.tile_pool(name="ph_lg", bufs=1))
        lrow_b = plg.tile([P, RJ, N], BF16)
        ccin_l = dram.tile([RJ * (NKC // 2) * P, FB], BF16, name="ccin_l")
        for j in range(RJ):
            for k in range(NKC):
                pe_t(lrow_b[:, j, k * P:(k + 1) * P],
                     l2tb[:, k, j * P:(j + 1) * P])
            for t in range(NKC // 2):
                nc.sync.dma_start(
                    ccin_l[(j * (NKC // 2) + t) * P:
                           (j * (NKC // 2) + t + 1) * P, :],
                    lrow_b[:, j, t * FB:(t + 1) * FB])
        lg = agather(ccin_l, "lg")
        plg_st.close()

        # =========================================================
        # Forcing: ft = P''-contract(x0) (+0.1*x0^T below); F gather
        # =========================================================
        slabp_st = ExitStack()
        slabp = slabp_st.enter_context(tc.tile_pool(name="slab_p", bufs=1))
        ft = lout.tile([P, DKC, FB], F32)

        def plain_pass(plain, rhs, evict, tag):
            for m in range(DKC):
                sl = slabp.tile([P, NKC, P], F32R, tag=tag, bufs=2,
                                name=f"slab_{tag}")
                nc.sync.dma_start(
                    sl[:], plain[:, m * P:(m + 1) * P].bitcast(F32R).rearrange(
                        "(k p) n -> p k n", p=P))
                ps = psum.tile([P, FB], F32, tag="mm0", bufs=2, name="ps_mm0")
                for k in range(NKC):
                    nc.tensor.matmul(ps[:], sl[:, k, :], rhs[:, k, :],
                                     start=(k == 0), stop=(k == NKC - 1))
                evict(m, ps)

        def ev_ft(m, ps):
            nc.vector.tensor_copy(ft[:, m, :], ps[:])
        plain_pass(x0_full, p2t, ev_ft, "icslab0")

        pf_st = ExitStack()
        pf = pf_st.enter_context(tc.tile_pool(name="ph_f", bufs=1))
        x0colT = pf.tile([P, DKC, FB], F32)
        xcolT = pf.tile([P, DKC, FB], F32)
        for srct, dst in ((x0_rows, x0colT), (x_rows, xcolT)):
            for j in range(RJ):
                rsb = pf.tile([P, D], F32, tag="rows_in", bufs=2, name="rows_in")
                nc.sync.dma_start(rsb[:], srct[j * P:(j + 1) * P, :])
                for m in range(DKC):
                    pe_t(dst[:, m, j * P:(j + 1) * P],
                         rsb[:, m * P:(m + 1) * P])
        for m in range(DKC):
            nc.vector.scalar_tensor_tensor(ft[:, m, :], x0colT[:, m, :], 0.1,
                                           ft[:, m, :], AL.mult, AL.add)

        # F gather (bf16 rows of F)
        frow_b = pf.tile([P, RJ, D], BF16)
        ccin_f = dram.tile([RJ * (DKC // 2) * P, FB], BF16, name="ccin_f")
        for j in range(RJ):
            for m in range(DKC):
                pe_t(frow_b[:, j, m * P:(m + 1) * P],
                     ft[:, m, j * P:(j + 1) * P])
            for mp in range(DKC // 2):
                nc.sync.dma_start(
                    ccin_f[(j * (DKC // 2) + mp) * P:
                           (j * (DKC // 2) + mp + 1) * P, :],
                    frow_b[:, j, mp * FB:(mp + 1) * FB])
        fg = agather(ccin_f, "fg")

        # --- step 0 V: from fp32 x directly ---
        v = lout.tile([P, DKC, FB], F32, tag="v", bufs=2, name="v")
        v_b = lout.tile([P, DKC, FB], BF16, tag="vb", bufs=2, name="v_b")

        def ev_v0(m, ps):
            nc.vector.tensor_add(v[:, m, :], ps[:], xcolT[:, m, :])
            nc.vector.tensor_copy(v_b[:, m, :], v[:, m, :])
        plain_pass(x_full, l2t, ev_v0, "icslab0")
        pf_st.close()
        slabp_st.close()
        pr_st.close()
        pa_st.close()

        # =========================================================
        # Recurrence pools + R'' slabs
        # =========================================================
        pe = top.enter_context(tc.tile_pool(name="ph_e", bufs=1))
        slabic = top.enter_context(tc.tile_pool(name="slab_ic", bufs=1))

        r2_sb = pe.tile([P, DKC, DKC // 2, FB], BF16)
        nc.sync.dma_start(
            r2_sb[:], r2_g[:].rearrange("(c t p) n -> p c t n",
                                        c=N_CORES, p=P))

        def r2_lhsT(k, m):
            return r2_sb[:, k, m // 2, (m % 2) * P:(m % 2 + 1) * P]

        def r_contract(dst, lhsT_fn, vv, vv_b, alpha, f_t):
            for m in range(DKC):
                ps = psum.tile([P, FB], F32, tag="mm1", bufs=2, name="ps_r")
                for k in range(DKC):
                    nc.tensor.matmul(ps[:], lhsT_fn(k, m), vv_b[:, k, :],
                                     start=(k == 0), stop=(k == DKC - 1))
                nc.vector.scalar_tensor_tensor(dst[:, m, :], vv[:, m, :],
                                               alpha, ps[:], AL.mult, AL.add)
                nc.vector.tensor_add(dst[:, m, :], dst[:, m, :], f_t[:, m, :])

        def ship(icnt, name):
            icrow_b = pe.tile([P, RJ, D], BF16, tag="icrow", bufs=2,
                              name="icrow_b")
            ccin = dram.tile([RJ * (DKC // 2) * P, FB], BF16, tag="ccin_ic",
                             bufs=2, name=f"ccin_{name}")
            for j in range(RJ):
                for m in range(DKC):
                    pe_t(icrow_b[:, j, m * P:(m + 1) * P],
                         icnt[:, m, j * P:(j + 1) * P])
                for mp in range(DKC // 2):
                    nc.sync.dma_start(
                        ccin[(j * (DKC // 2) + mp) * P:
                             (j * (DKC // 2) + mp + 1) * P, :],
                        icrow_b[:, j, mp * FB:(mp + 1) * FB])
            return agather(ccin, name)

        def v_pass(g, rhs_b, vv, vv_b, prev):
            """vv = Full(g)^T-contract with rhs_b, + prev (exact fp32 term)."""
            ga = g[:].rearrange("(c j t2 p) n -> p c j t2 n",
                                c=N_CORES, j=RJ, t2=DKC // 2, p=P)
            for mp in range(DKC // 2):
                sl = slabic.tile([P, N_CORES, RJ, FB], BF16, tag="icslab",
                                 bufs=3, name="slab_ic")
                nc.sync.dma_start(sl[:], ga[:, :, :, mp, :])
                for mh in range(2):
                    m = mp * 2 + mh
                    ps = psum.tile([P, FB], F32, tag="mm0", bufs=2,
                                   name="ps_mm0")
                    nk = 0
                    for c in range(N_CORES):
                        for j in range(RJ):
                            nc.tensor.matmul(
                                ps[:], sl[:, c, j, mh * P:(mh + 1) * P],
                                rhs_b[:, c * RJ + j, :],
                                start=(nk == 0), stop=(nk == NKC - 1))
                            nk += 1
                    nc.vector.tensor_add(vv[:, m, :], ps[:], prev[:, m, :])
                    nc.vector.tensor_copy(vv_b[:, m, :], vv[:, m, :])

        # --- step 0 R-contract + ship ---
        icnt = pe.tile([P, DKC, FB], F32, tag="icnt", bufs=2, name="icnt")
        r_contract(icnt, r2_lhsT, v, v_b, ABR, ft)
        s0_g = ship(icnt, "s0")
        icnt_prev = icnt

        # --- Phi^2 precompute (covers the s0 gather) ---
        # R4 = 2a R'' + R''^2  (local pass off r2_sb)
        r4_full = pe.tile([P, DKC, D], BF16)
        for m in range(DKC):
            for c4 in range(DKC // 2):
                ps = psum.tile([P, FB], F32, tag="mm1", bufs=2, name="ps_r")
                for k in range(DKC):
                    nc.tensor.matmul(ps[:], r2_lhsT(k, m),
                                     r2_sb[:, k, c4, :],
                                     start=(k == 0), stop=(k == DKC - 1))
                sc = scrp.tile([P, FB], F32, tag="r4c", bufs=3, name="r4c")
                nc.vector.tensor_copy(sc[:], r2_sb[:, m, c4, :])
                nc.vector.scalar_tensor_tensor(sc[:], sc[:], 2.0 * ABR,
                                               ps[:], AL.mult, AL.add)
                nc.vector.tensor_copy(r4_full[:, m, c4 * FB:(c4 + 1) * FB],
                                      sc[:])

        def r4_lhsT(k, m):
            return r4_full[:, k, m * P:(m + 1) * P]

        # M2: L4 = 2L' + L'^2 (npass over the single L' gather)
        l4tb = pe.tile([P, NKC, FB], BF16)
        la = lg[:].rearrange("(c j t p) n -> p c j t n",
                             c=N_CORES, j=RJ, t=NKC // 2, p=P)
        for mp in range(NKC // 2):
            sl = slabic.tile([P, N_CORES, RJ, FB], BF16, tag="icslab",
                             bufs=3, name="slab_ic")
            nc.sync.dma_start(sl[:], la[:, :, :, mp, :])
            for mh in range(2):
                m = mp * 2 + mh
                ps = psum.tile([P, FB], F32, tag="mm0", bufs=2, name="ps_mm0")
                nk = 0
                for j in range(RJ):
                    for c in range(N_CORES):
                        nc.tensor.matmul(ps[:], sl[:, c, j, mh * P:(mh + 1) * P],
                                         l2tb[:, c * RJ + j, :],
                                         start=(nk == 0), stop=(nk == NKC - 1))
                        nk += 1
                sc = scrp.tile([P, FB], F32, tag="l4c", bufs=3, name="l4c")
                nc.vector.scalar_tensor_tensor(
                    sc[:], l2t[:, m, :].bitcast(F32), 2.0, ps[:],
                    AL.mult, AL.add)
                nc.vector.tensor_copy(l4tb[:, m, :], sc[:])

        # F2 = a*(F + L'F) + (F + L'F)@R'' + F : U = L'@F_gathered + F
        u = lout.tile([P, DKC, FB], F32, tag="v", bufs=2, name="v")
        u_b = lout.tile([P, DKC, FB], BF16, tag="vb", bufs=2, name="v_b")
        v_pass(fg, l2tb, u, u_b, ft)
        ft2 = pe.tile([P, DKC, FB], F32)
        r_contract(ft2, r2_lhsT, u, u_b, ABR, ft)

        # --- 4 double steps ---
        g_prev = s0_g
        A2 = ABR * ABR
        for dstep in range(4):
            vv = lout.tile([P, DKC, FB], F32, tag="v", bufs=2, name="v")
            vv_b = lout.tile([P, DKC, FB], BF16, tag="vb", bufs=2, name="v_b")
            v_pass(g_prev, l4tb, vv, vv_b, icnt_prev)
            icnt = pe.tile([P, DKC, FB], F32, tag="icnt", bufs=2, name="icnt")
            r_contract(icnt, r4_lhsT, vv, vv_b, A2, ft2)
            if dstep < 3:
                g_prev = ship(icnt, f"d{dstep}")
                icnt_prev = icnt
            else:
                icrow_f = pe.tile([P, RJ, D], F32, name="icrow_f")
                for j in range(RJ):
                    for m in range(DKC):
                        pe_t(icrow_f[:, j, m * P:(m + 1) * P],
                             icnt[:, m, j * P:(j + 1) * P])
                    nc.sync.dma_start(z_loc[j * P:(j + 1) * P, :],
                                      icrow_f[:, j, :])

    nc.compile()
    return nc


_NC_CACHE = []


def _get_nc():
    if not _NC_CACHE:
        _NC_CACHE.append(build_nc())
    return _NC_CACHE[0]


def make_in_maps(inputs):
    x = np.ascontiguousarray(np.asarray(inputs["x"], dtype=np.float32))
    x0 = np.ascontiguousarray(np.asarray(inputs["x0"], dtype=np.float32))
    adj = np.asarray(inputs["adj"], dtype=np.float32)
    alpha = np.ascontiguousarray(np.asarray(inputs["alpha_train"],
                                            dtype=np.float32))
    w = np.asarray(inputs["w"], dtype=np.float32)
    d = np.ascontiguousarray(np.asarray(inputs["d"], dtype=np.float32))

    am = adj - np.eye(N, dtype=np.float32)
    wT = np.ascontiguousarray(w.T)

    in_maps = []
    for c in range(N_CORES):
        r0 = c * RB
        f0 = c * FBR
        in_maps.append({
            "am_rows": np.ascontiguousarray(am[r0:r0 + RB, :]),
            "alpha_blk": np.ascontiguousarray(alpha[r0:r0 + RB]),
            "x_full": x,
            "x0_full": x0,
            "x_rows": np.ascontiguousarray(x[r0:r0 + RB, :]),
            "x0_rows": np.ascontiguousarray(x0[r0:r0 + RB, :]),
            "wT_full": wT,
            "wTc": np.ascontiguousarray(wT[:, f0:f0 + FBR]),
            "d_full": d,
        })
    return in_maps


def kernel(**inputs) -> np.ndarray:
    nc = _get_nc()
    in_maps = make_in_maps(inputs)
    res = run_bass_kernel_spmd(nc, in_maps, core_ids=list(range(N_CORES)))
    z = np.concatenate([res.results[c]["z_loc"] for c in range(N_CORES)], axis=0)
    return np.ascontiguousarray(z.astype(np.float32))


if __name__ == "__main__":
    rng = np.random.default_rng(0)
    ins = {
        "x": rng.standard_normal((N, D)).astype(np.float32),
        "x0": rng.standard_normal((N, D)).astype(np.float32),
        "adj": (rng.random((N, N)) / N).astype(np.float32),
        "alpha_train": rng.standard_normal((N,)).astype(np.float32),
        "w": (np.eye(D) + 0.02 * rng.standard_normal((D, D))).astype(np.float32),
        "d": rng.random((D,)).astype(np.float32),
    }
    out = kernel(**ins)
    print("kernel output:", out.shape, out.dtype, float(np.linalg.norm(out)))
